# revision 11
# baseline (speedup 1.0000x reference)
"""Allegro GNN edge-update layer on 8 TRN2 NeuronCores.

Strategy (per sharding hint): shard edges across 8 cores, replicate the
small MLP/LN params. The per-edge node-feature gather is pure data
movement, done host-side while laying out shards (all on-device gather
primitives are Q7-descriptor-rate limited, ~8-28ns/row = 3.4ms+/core —
far over the ~0.4ms memory roofline). The device kernel does all the
arithmetic: 3-layer MLP (bf16 matmuls, fp32 PSUM), SiLU, residual add,
LayerNorm.

Device layouts (per core, E_c edges padded to a multiple of 2048):
  xa  bf16 [128, E_c]  rows 0:64 node_feats[row].T, rows 64:128 edge_feats.T
  xb  bf16 [80,  E_c]  rows 0:64 node_feats[col].T, rows 64:80 edge_rbf.T
  out f32  [128, E_c/128, 64]  out[p, b, :] = y[b*128 + p, :]  (un-swizzled on host)

Per 2048-edge macro-tile (quarters q=0..3 of 512 edges; quarter q maps to
PSUM partitions 64*(q%2), free 512*(q//2)):
  PE:   h1 = W1a.T xa + W1b.T xb (K=128+80), h2 = W2.T silu(h1+b1),
        h3 = W3.T silu(h2+b2); then per 128-edge block j an augmented
        transpose-matmul lhsT=[upd; efT] rhs=[I64;I64|ones] producing
        y = upd+ef (edge-major) AND S = sum_f(y) in one PSUM write.
  ACT:  silu1, silu2, h3+b3 evictions (even quarters in-place into xa's
        dead niT rows; odd quarters to a bounce tile).
  DMA:  shifts odd-quarter upd from partitions 64:128 down to xa rows 0:64
        (HW: transpose matmuls require base partition 0 sources).
  DVE:  mu = S/64, t = y - mu (broadcast), var reduce, reciprocal.
  Pool: tsq = t*t, out = t * rs (broadcast).
  Sqrt runs on ACT once per 8 macros (Silu and Sqrt live in different
  activation-table sets; batching amortizes the ~1.3us table reloads).
"""
import sys

sys.path.insert(0, '/opt/trn_rl_repo')

import numpy as np
import ml_dtypes

import concourse.bass as bass
import concourse.bacc as bacc
import concourse.tile as tile
from concourse import mybir
from concourse.bass_utils import run_bass_kernel_spmd

BF16 = ml_dtypes.bfloat16
N_CORES = 8
MACRO = 2048                # edges per macro-tile
NQ = MACRO // 512           # 512-edge quarters per macro
NBLK = MACRO // 128         # 128-edge transpose blocks per macro
GB = 8                      # macros per sqrt batch

_cache = {}
_last_in_maps = None


def _build(e_core, n_macro, identity_ln):
    nc = bacc.Bacc("TRN2", target_bir_lowering=False, debug=False,
                   num_devices=N_CORES)
    f32, bf16 = mybir.dt.float32, mybir.dt.bfloat16
    ACT = mybir.ActivationFunctionType

    xa_d = nc.declare_dram_parameter("xa", [128, e_core], bf16, isOutput=False)
    xb_d = nc.declare_dram_parameter("xb", [80, e_core], bf16, isOutput=False)
    w1a_d = nc.declare_dram_parameter("w1a", [128, 64], bf16, isOutput=False)
    w1b_d = nc.declare_dram_parameter("w1b", [80, 64], bf16, isOutput=False)
    w2_d = nc.declare_dram_parameter("w2", [128, 64], bf16, isOutput=False)
    w3_d = nc.declare_dram_parameter("w3", [128, 64], bf16, isOutput=False)
    iio_d = nc.declare_dram_parameter("iio", [128, 65], bf16, isOutput=False)
    brep_d = nc.declare_dram_parameter("brep", [128, 3], f32, isOutput=False)
    gb_d = nc.declare_dram_parameter("gb", [2, 64], f32, isOutput=False)
    out_d = nc.declare_dram_parameter("out", [128, e_core // 128, 64],
                                      f32, isOutput=True)

    with tile.TileContext(nc) as tc:
        with tc.tile_pool(name="singles", bufs=1) as singles, \
             tc.tile_pool(name="ins", bufs=4) as ins, \
             tc.tile_pool(name="acts", bufs=3) as acts, \
             tc.tile_pool(name="ts", bufs=2 * GB + 2) as ts, \
             tc.tile_pool(name="outs", bufs=4) as outs, \
             tc.tile_pool(name="stats", bufs=2 * GB + 2) as stats, \
             tc.tile_pool(name="bstats", bufs=2) as bstats, \
             tc.tile_pool(name="ph", bufs=2, space="PSUM") as ph, \
             tc.tile_pool(name="pu", bufs=1, space="PSUM") as pu:

            w1a = singles.tile([128, 64], bf16)
            nc.sync.dma_start(out=w1a[:], in_=w1a_d[:, :])
            w1b = singles.tile([80, 64], bf16)
            nc.sync.dma_start(out=w1b[:], in_=w1b_d[:, :])
            w2 = singles.tile([128, 64], bf16)
            nc.sync.dma_start(out=w2[:], in_=w2_d[:, :])
            w3 = singles.tile([128, 64], bf16)
            nc.sync.dma_start(out=w3[:], in_=w3_d[:, :])
            iio = singles.tile([128, 65], bf16)
            nc.sync.dma_start(out=iio[:], in_=iio_d[:, :])
            brep = singles.tile([128, 3], f32)
            nc.sync.dma_start(out=brep[:], in_=brep_d[:, :])
            eps = singles.tile([128, 1], f32)
            nc.vector.memset(eps[:], 1e-5)
            zeros16 = singles.tile([128, 16], f32)
            nc.vector.memset(zeros16[:], 0.0)
            if not identity_ln:
                gamma_t = singles.tile([128, 64], f32)
                nc.gpsimd.dma_start(
                    out=gamma_t[:],
                    in_=bass.AP(tensor=gb_d, offset=0, ap=[[0, 128], [1, 64]]))
                beta_t = singles.tile([128, 64], f32)
                nc.gpsimd.dma_start(
                    out=beta_t[:],
                    in_=bass.AP(tensor=gb_d, offset=64, ap=[[0, 128], [1, 64]]))

            pend = []   # (t, t_tile, slot) awaiting batched rs

            def flush(var8, rs8):
                if not pend:
                    return
                nb = pend[-1][2] + 1
                nc.scalar.activation(var8[:, :nb, :], var8[:, :nb, :],
                                     ACT.Sqrt, bias=eps[:], scale=1.0)
                nc.vector.reciprocal(out=rs8[:, :nb, :], in_=var8[:, :nb, :])
                for (t, t_t, sl) in pend:
                    rs_b = rs8[:, sl, :, None].to_broadcast([128, 16, 64])
                    o = outs.tile([128, NBLK, 64], f32, tag="o")
                    nc.gpsimd.tensor_tensor(out=o[:], in0=t_t[:], in1=rs_b,
                                            op=mybir.AluOpType.mult)
                    if not identity_ln:
                        gb_ap = gamma_t[:, None, :].to_broadcast([128, NBLK, 64])
                        bb_ap = beta_t[:, None, :].to_broadcast([128, NBLK, 64])
                        nc.vector.tensor_tensor(out=o[:], in0=o[:], in1=gb_ap,
                                                op=mybir.AluOpType.mult)
                        nc.vector.tensor_tensor(out=o[:], in0=o[:], in1=bb_ap,
                                                op=mybir.AluOpType.add)
                    nc.sync.dma_start(out=out_d[:, t * NBLK:(t + 1) * NBLK, :],
                                      in_=o[:])
                pend.clear()

            var8 = rs8 = None
            for t in range(n_macro):
                slot = t % GB
                if slot == 0:
                    var8 = bstats.tile([128, GB, 16], f32, tag="v8")
                    rs8 = bstats.tile([128, GB, 16], f32, tag="r8")
                e0 = t * MACRO
                xa_t = ins.tile([128, MACRO], bf16, tag="xa")
                nc.sync.dma_start(out=xa_t[:], in_=xa_d[:, e0:e0 + MACRO])
                xb_t = ins.tile([80, MACRO], bf16, tag="xb")
                nc.sync.dma_start(out=xb_t[:], in_=xb_d[:, e0:e0 + MACRO])

                # ---- layer 1 ----
                h1p = ph.tile([128, MACRO // 2], f32, tag="h", space="PSUM")
                for q in range(NQ):
                    pr, fr = 64 * (q % 2), 512 * (q // 2)
                    o = h1p[pr:pr + 64, fr:fr + 512]
                    nc.tensor.matmul(o, lhsT=w1a[:], rhs=xa_t[:, 512 * q:512 * q + 512],
                                     start=True, stop=False)
                    nc.tensor.matmul(o, lhsT=w1b[:], rhs=xb_t[:, 512 * q:512 * q + 512],
                                     start=False, stop=True)
                h1s = acts.tile([128, MACRO // 2], bf16, tag="h1s")
                nc.scalar.activation(h1s[:], h1p[:], ACT.Silu,
                                     bias=brep[:, 0:1], scale=1.0)

                # ---- layer 2 ----
                h2p = ph.tile([128, MACRO // 2], f32, tag="h", space="PSUM")
                for q in range(NQ):
                    pr, fr = 64 * (q % 2), 512 * (q // 2)
                    nc.tensor.matmul(h2p[pr:pr + 64, fr:fr + 512], lhsT=w2[pr:pr + 64, :],
                                     rhs=h1s[pr:pr + 64, fr:fr + 512],
                                     start=True, stop=True)
                h2s = acts.tile([128, MACRO // 2], bf16, tag="h2s")
                nc.scalar.activation(h2s[:], h2p[:], ACT.Silu,
                                     bias=brep[:, 1:2], scale=1.0)

                # ---- layer 3 ----
                h3p = ph.tile([128, MACRO // 2], f32, tag="h", space="PSUM")
                for q in range(NQ):
                    pr, fr = 64 * (q % 2), 512 * (q // 2)
                    nc.tensor.matmul(h3p[pr:pr + 64, fr:fr + 512], lhsT=w3[pr:pr + 64, :],
                                     rhs=h2s[pr:pr + 64, fr:fr + 512],
                                     start=True, stop=True)

                # ---- evict upd = h3+b3 into xa's dead niT rows (feature-major) ----
                xa2 = xa_t[0:64, :].rearrange("p (a b) -> p a b", b=1024)
                # even quarters (q0,q2): h3p partitions 0:64 -> xa col-halves [i, 0:512]
                nc.scalar.activation(
                    xa2[:, :, 0:512],
                    h3p[0:64, :].rearrange("p (a b) -> p a b", b=512),
                    ACT.Identity, bias=brep[0:64, 2:3], scale=1.0)
                # odd quarters (q1,q3): partitions 64:128 -> bounce + DMA shift down
                updB = acts.tile([128, MACRO // 2], bf16, tag="updB")
                nc.scalar.activation(
                    updB[64:128, :], h3p[64:128, :],
                    ACT.Identity, bias=brep[64:128, 2:3], scale=1.0)
                nc.sync.dma_start(
                    out=xa2[:, :, 512:1024],
                    in_=updB[64:128, :].rearrange("p (a b) -> p a b", b=512))

                # ---- augmented transposes ----
                # yS[:, j//8, j%8, 0:64] = y block j (edge-major); [..., 64] = sum_f y
                yS = pu.tile([128, 4, 512], f32, tag="u", space="PSUM")
                for j in range(NBLK):
                    nc.tensor.matmul(yS[:, j // 4, (j % 4) * 65:(j % 4) * 65 + 65],
                                     lhsT=xa_t[:, 128 * j:128 * j + 128],
                                     rhs=iio[:], start=True, stop=True)
                ySv = yS[:, :, 0:260].rearrange("p a (b c) -> p a b c", c=65)

                # mu = S/64
                mu = stats.tile([128, 4, 4], f32, tag="mu")
                nc.vector.scalar_tensor_tensor(
                    out=mu[:], in0=ySv[:, :, :, 64], scalar=1.0 / 64,
                    in1=zeros16[:].rearrange("p (a b) -> p a b", a=4),
                    op0=mybir.AluOpType.mult, op1=mybir.AluOpType.add)

                # t = y - mu (broadcast)  [DVE, PSUM read]
                t_t = ts.tile([128, NBLK, 64], bf16, tag="t")
                mu_b = mu[:, :, :, None].to_broadcast([128, 4, 4, 64])
                nc.vector.tensor_tensor(
                    out=t_t[:].rearrange("p (a b) c -> p a b c", a=4),
                    in0=ySv[:, :, :, 0:64], in1=mu_b,
                    op=mybir.AluOpType.subtract)

                # tsq = t*t [Pool]; var = sum(tsq)/64 [DVE]
                tsq = ts.tile([128, NBLK, 64], bf16, tag="tsq")
                nc.gpsimd.tensor_tensor(out=tsq[:], in0=t_t[:], in1=t_t[:],
                                        op=mybir.AluOpType.mult)
                q_t = stats.tile([128, 16], f32, tag="q")
                nc.vector.tensor_reduce(out=q_t[:], in_=tsq[:],
                                        axis=mybir.AxisListType.X,
                                        op=mybir.AluOpType.add)
                nc.vector.scalar_tensor_tensor(
                    out=var8[:, slot, :], in0=q_t[:], scalar=1.0 / 64,
                    in1=zeros16[:],
                    op0=mybir.AluOpType.mult, op1=mybir.AluOpType.add)

                pend.append((t, t_t, slot))
                if slot == GB - 1 or t == n_macro - 1:
                    flush(var8, rs8)

    nc.compile()
    return nc


def kernel(edge_feats, node_feats, edge_index, edge_rbf,
           W1, b1, W2, b2, W3, b3, ln_gamma, ln_beta):
    global _last_in_maps
    E = edge_feats.shape[0]
    e_core = -(-E // (N_CORES * MACRO)) * MACRO     # per-core edges, padded
    n_macro = e_core // MACRO
    e_pad = e_core * N_CORES

    identity_ln = bool(np.allclose(ln_gamma, 1.0) and np.allclose(ln_beta, 0.0))
    key = (e_core, identity_ln)
    if key not in _cache:
        _cache[key] = _build(e_core, n_macro, identity_ln)
    nc = _cache[key]

    row = np.asarray(edge_index[0]).astype(np.int64)
    col = np.asarray(edge_index[1]).astype(np.int64)
    ef32 = np.asarray(edge_feats, dtype=np.float32)
    nf_bf = np.asarray(node_feats, dtype=np.float32).astype(BF16)
    ef_bf = ef32.astype(BF16)
    rbf_bf = np.asarray(edge_rbf, dtype=np.float32).astype(BF16)

    xa = np.zeros((128, e_pad), dtype=BF16)
    xa[0:64, :E] = nf_bf[row].T
    xa[64:128, :E] = ef_bf.T
    xb = np.zeros((80, e_pad), dtype=BF16)
    xb[0:64, :E] = nf_bf[col].T
    xb[64:80, :E] = rbf_bf.T

    W1f = np.asarray(W1, np.float32)
    w1a = np.ascontiguousarray(
        np.concatenate([W1f[64:128], W1f[0:64]], axis=0)).astype(BF16)
    w1b = np.ascontiguousarray(W1f[128:208]).astype(BF16)
    w2 = np.tile(np.asarray(W2, np.float32).astype(BF16), (2, 1))
    w3 = np.tile(np.asarray(W3, np.float32).astype(BF16), (2, 1))
    iio = np.zeros((128, 65), dtype=np.float32)
    iio[0:64, 0:64] = np.eye(64)
    iio[64:128, 0:64] = np.eye(64)
    iio[:, 64] = 1.0
    iio = iio.astype(BF16)
    brep = np.stack([np.tile(np.asarray(v, np.float32), 2) for v in (b1, b2, b3)],
                    axis=1)                                    # [128, 3]
    gb = np.stack([np.asarray(ln_gamma, np.float32),
                   np.asarray(ln_beta, np.float32)])           # [2, 64]

    in_maps = []
    for c in range(N_CORES):
        s = slice(c * e_core, (c + 1) * e_core)
        in_maps.append({
            "xa": np.ascontiguousarray(xa[:, s]),
            "xb": np.ascontiguousarray(xb[:, s]),
            "w1a": w1a, "w1b": w1b, "w2": w2, "w3": w3,
            "iio": iio, "brep": brep, "gb": gb,
        })

    _last_in_maps = in_maps
    res = run_bass_kernel_spmd(nc, in_maps, core_ids=list(range(N_CORES)))

    out = np.empty((e_pad, 64), dtype=np.float32)
    for c in range(N_CORES):
        blk = res.results[c]["out"].transpose(1, 0, 2).reshape(e_core, 64)
        out[c * e_core:(c + 1) * e_core] = blk
    return out[:E]


# revision 12
# speedup vs baseline: 1.0515x; 1.0515x over previous
"""Allegro GNN edge-update layer on 8 TRN2 NeuronCores.

Strategy (per sharding hint): shard edges across 8 cores, replicate the
small MLP/LN params. The per-edge node-feature gather is pure data
movement, done host-side while laying out shards (all on-device gather
primitives are Q7-descriptor-rate limited, ~8-28ns/row = 3.4ms+/core —
far over the ~0.4ms memory roofline). The device kernel does all the
arithmetic: 3-layer MLP (bf16 matmuls, fp32 PSUM), SiLU, residual add,
LayerNorm.

Device layouts (per core, E_c edges padded to a multiple of 2048):
  xa  bf16 [128, E_c]  rows 0:64 node_feats[row].T, rows 64:128 edge_feats.T
  xb  bf16 [80,  E_c]  rows 0:64 node_feats[col].T, rows 64:80 edge_rbf.T
  out f32  [128, E_c/128, 64]  out[p, b, :] = y[b*128 + p, :]  (un-swizzled on host)

Per 2048-edge macro-tile (quarters q=0..3 of 512 edges; quarter q maps to
PSUM partitions 64*(q%2), free 512*(q//2)):
  PE:   h1 = W1a.T xa + W1b.T xb (K=128+80), h2 = W2.T silu(h1+b1),
        h3 = W3.T silu(h2+b2); then per 128-edge block j an augmented
        transpose-matmul lhsT=[upd; efT] rhs=[I64;I64|ones] producing
        y = upd+ef (edge-major) AND S = sum_f(y) in one PSUM write.
  ACT:  silu1, silu2, h3+b3 evictions (even quarters in-place into xa's
        dead niT rows; odd quarters to a bounce tile).
  DMA:  shifts odd-quarter upd from partitions 64:128 down to xa rows 0:64
        (HW: transpose matmuls require base partition 0 sources).
  DVE:  mu = S/64, t = y - mu (broadcast), var reduce, reciprocal.
  Pool: tsq = t*t, out = t * rs (broadcast).
  Sqrt runs on ACT once per 8 macros (Silu and Sqrt live in different
  activation-table sets; batching amortizes the ~1.3us table reloads).
"""
import sys

sys.path.insert(0, '/opt/trn_rl_repo')

import numpy as np
import ml_dtypes

import concourse.bass as bass
import concourse.bacc as bacc
import concourse.tile as tile
from concourse import mybir
from concourse.bass_utils import run_bass_kernel_spmd

BF16 = ml_dtypes.bfloat16
N_CORES = 8
MACRO = 2048                # edges per macro-tile
NQ = MACRO // 512           # 512-edge quarters per macro
NBLK = MACRO // 128         # 128-edge transpose blocks per macro
GB = 8                      # macros per sqrt batch

_cache = {}
_last_in_maps = None


def _build(e_core, n_macro, identity_ln):
    nc = bacc.Bacc("TRN2", target_bir_lowering=False, debug=False,
                   num_devices=N_CORES)
    f32, bf16 = mybir.dt.float32, mybir.dt.bfloat16
    ACT = mybir.ActivationFunctionType

    xa_d = nc.declare_dram_parameter("xa", [128, e_core], bf16, isOutput=False)
    xb_d = nc.declare_dram_parameter("xb", [80, e_core], bf16, isOutput=False)
    w1a_d = nc.declare_dram_parameter("w1a", [128, 64], bf16, isOutput=False)
    w1b_d = nc.declare_dram_parameter("w1b", [80, 64], bf16, isOutput=False)
    w2_d = nc.declare_dram_parameter("w2", [128, 64], bf16, isOutput=False)
    w3_d = nc.declare_dram_parameter("w3", [128, 64], bf16, isOutput=False)
    iio_d = nc.declare_dram_parameter("iio", [128, 65], bf16, isOutput=False)
    brep_d = nc.declare_dram_parameter("brep", [128, 3], f32, isOutput=False)
    gb_d = nc.declare_dram_parameter("gb", [2, 64], f32, isOutput=False)
    out_d = nc.declare_dram_parameter("out", [128, e_core // 128, 64],
                                      f32, isOutput=True)

    with tile.TileContext(nc) as tc:
        with tc.tile_pool(name="singles", bufs=1) as singles, \
             tc.tile_pool(name="ins", bufs=4) as ins, \
             tc.tile_pool(name="acts", bufs=3) as acts, \
             tc.tile_pool(name="ts", bufs=2 * GB + 2) as ts, \
             tc.tile_pool(name="outs", bufs=4) as outs, \
             tc.tile_pool(name="stats", bufs=2 * GB + 2) as stats, \
             tc.tile_pool(name="bstats", bufs=2) as bstats, \
             tc.tile_pool(name="ph", bufs=3, space="PSUM") as ph, \
             tc.tile_pool(name="pu", bufs=1, space="PSUM") as pu:

            w1a = singles.tile([128, 64], bf16)
            nc.sync.dma_start(out=w1a[:], in_=w1a_d[:, :])
            w1b = singles.tile([80, 64], bf16)
            nc.sync.dma_start(out=w1b[:], in_=w1b_d[:, :])
            w2 = singles.tile([128, 64], bf16)
            nc.sync.dma_start(out=w2[:], in_=w2_d[:, :])
            w3 = singles.tile([128, 64], bf16)
            nc.sync.dma_start(out=w3[:], in_=w3_d[:, :])
            iio = singles.tile([128, 65], bf16)
            nc.sync.dma_start(out=iio[:], in_=iio_d[:, :])
            brep = singles.tile([128, 3], f32)
            nc.sync.dma_start(out=brep[:], in_=brep_d[:, :])
            eps = singles.tile([128, 1], f32)
            nc.vector.memset(eps[:], 1e-5)
            zeros16 = singles.tile([128, 16], f32)
            nc.vector.memset(zeros16[:], 0.0)
            if not identity_ln:
                gamma_t = singles.tile([128, 64], f32)
                nc.gpsimd.dma_start(
                    out=gamma_t[:],
                    in_=bass.AP(tensor=gb_d, offset=0, ap=[[0, 128], [1, 64]]))
                beta_t = singles.tile([128, 64], f32)
                nc.gpsimd.dma_start(
                    out=beta_t[:],
                    in_=bass.AP(tensor=gb_d, offset=64, ap=[[0, 128], [1, 64]]))

            pend = []   # (t, t_tile, slot) awaiting batched rs

            def flush(var8, rs8):
                if not pend:
                    return
                nb = pend[-1][2] + 1
                nc.scalar.activation(var8[:, :nb, :], var8[:, :nb, :],
                                     ACT.Sqrt, bias=eps[:], scale=1.0)
                nc.vector.reciprocal(out=rs8[:, :nb, :], in_=var8[:, :nb, :])
                for (t, t_t, sl) in pend:
                    rs_b = rs8[:, sl, :, None].to_broadcast([128, 16, 64])
                    o = outs.tile([128, NBLK, 64], f32, tag="o")
                    nc.gpsimd.tensor_tensor(out=o[:], in0=t_t[:], in1=rs_b,
                                            op=mybir.AluOpType.mult)
                    if not identity_ln:
                        gb_ap = gamma_t[:, None, :].to_broadcast([128, NBLK, 64])
                        bb_ap = beta_t[:, None, :].to_broadcast([128, NBLK, 64])
                        nc.vector.tensor_tensor(out=o[:], in0=o[:], in1=gb_ap,
                                                op=mybir.AluOpType.mult)
                        nc.vector.tensor_tensor(out=o[:], in0=o[:], in1=bb_ap,
                                                op=mybir.AluOpType.add)
                    nc.sync.dma_start(out=out_d[:, t * NBLK:(t + 1) * NBLK, :],
                                      in_=o[:])
                pend.clear()

            var8 = rs8 = None
            for t in range(n_macro):
                slot = t % GB
                if slot == 0:
                    var8 = bstats.tile([128, GB, 16], f32, tag="v8")
                    rs8 = bstats.tile([128, GB, 16], f32, tag="r8")
                e0 = t * MACRO
                xa_t = ins.tile([128, MACRO], bf16, tag="xa")
                nc.sync.dma_start(out=xa_t[:], in_=xa_d[:, e0:e0 + MACRO])
                xb_t = ins.tile([80, MACRO], bf16, tag="xb")
                nc.sync.dma_start(out=xb_t[:], in_=xb_d[:, e0:e0 + MACRO])

                # ---- layer 1 ----
                h1p = ph.tile([128, MACRO // 2], f32, tag="h", space="PSUM")
                for q in range(NQ):
                    pr, fr = 64 * (q % 2), 512 * (q // 2)
                    o = h1p[pr:pr + 64, fr:fr + 512]
                    nc.tensor.matmul(o, lhsT=w1a[:], rhs=xa_t[:, 512 * q:512 * q + 512],
                                     start=True, stop=False)
                    nc.tensor.matmul(o, lhsT=w1b[:], rhs=xb_t[:, 512 * q:512 * q + 512],
                                     start=False, stop=True)
                h1s = acts.tile([128, MACRO // 2], bf16, tag="h1s")
                nc.scalar.activation(h1s[:], h1p[:], ACT.Silu,
                                     bias=brep[:, 0:1], scale=1.0)

                # ---- layer 2 ----
                h2p = ph.tile([128, MACRO // 2], f32, tag="h", space="PSUM")
                for q in range(NQ):
                    pr, fr = 64 * (q % 2), 512 * (q // 2)
                    nc.tensor.matmul(h2p[pr:pr + 64, fr:fr + 512], lhsT=w2[pr:pr + 64, :],
                                     rhs=h1s[pr:pr + 64, fr:fr + 512],
                                     start=True, stop=True)
                h2s = acts.tile([128, MACRO // 2], bf16, tag="h2s")
                nc.scalar.activation(h2s[:], h2p[:], ACT.Silu,
                                     bias=brep[:, 1:2], scale=1.0)

                # ---- layer 3 ----
                h3p = ph.tile([128, MACRO // 2], f32, tag="h", space="PSUM")
                for q in range(NQ):
                    pr, fr = 64 * (q % 2), 512 * (q // 2)
                    nc.tensor.matmul(h3p[pr:pr + 64, fr:fr + 512], lhsT=w3[pr:pr + 64, :],
                                     rhs=h2s[pr:pr + 64, fr:fr + 512],
                                     start=True, stop=True)

                # ---- evict upd = h3+b3 into xa's dead niT rows (feature-major) ----
                xa2 = xa_t[0:64, :].rearrange("p (a b) -> p a b", b=1024)
                # even quarters (q0,q2): h3p partitions 0:64 -> xa col-halves [i, 0:512]
                nc.scalar.activation(
                    xa2[:, :, 0:512],
                    h3p[0:64, :].rearrange("p (a b) -> p a b", b=512),
                    ACT.Identity, bias=brep[0:64, 2:3], scale=1.0)
                # odd quarters (q1,q3): stay at partitions 64:128 in a bounce tile
                updB = acts.tile([128, MACRO // 2], bf16, tag="updB")
                nc.scalar.activation(
                    updB[64:128, :], h3p[64:128, :],
                    ACT.Identity, bias=brep[64:128, 2:3], scale=1.0)


                # ---- augmented transposes ----
                # yS[:, j//8, j%8, 0:64] = y block j (edge-major); [..., 64] = sum_f y
                yS = pu.tile([128, NBLK, 64], f32, tag="u", space="PSUM")
                for j in range(NBLK):
                    q = j // 4
                    if q % 2 == 0:
                        # upd lives in xa rows 0:64 (in-place evict), ef in 64:128
                        nc.tensor.matmul(yS[:, j, :],
                                         lhsT=xa_t[:, 128 * j:128 * j + 128],
                                         rhs=iio[:, 0:64], start=True, stop=True)
                    else:
                        # odd quarters: upd in bounce tile at partitions 64:128
                        oc = 512 * (q // 2) + 128 * (j % 4)
                        nc.tensor.matmul(yS[:, j, :],
                                         lhsT=updB[64:128, oc:oc + 128],
                                         rhs=iio[64:128, 0:64], start=True, stop=False)
                        nc.tensor.matmul(yS[:, j, :],
                                         lhsT=xa_t[64:128, 128 * j:128 * j + 128],
                                         rhs=iio[64:128, 0:64], start=False, stop=True)

                # mu = S/64
                s_t = stats.tile([128, 16], f32, tag="s")
                nc.vector.tensor_reduce(out=s_t[:], in_=yS[:],
                                        axis=mybir.AxisListType.X,
                                        op=mybir.AluOpType.add)
                mu = stats.tile([128, 16], f32, tag="mu")
                nc.vector.scalar_tensor_tensor(
                    out=mu[:], in0=s_t[:], scalar=1.0 / 64,
                    in1=zeros16[:],
                    op0=mybir.AluOpType.mult, op1=mybir.AluOpType.add)

                # t = y - mu (broadcast)  [DVE, PSUM read]
                t_t = ts.tile([128, NBLK, 64], bf16, tag="t")
                mu_b = mu[:, :, None].to_broadcast([128, NBLK, 64])
                nc.vector.tensor_tensor(
                    out=t_t[:], in0=yS[:], in1=mu_b,
                    op=mybir.AluOpType.subtract)

                # tsq = t*t [Pool]; var = sum(tsq)/64 [DVE]
                tsq = ts.tile([128, NBLK, 64], bf16, tag="tsq")
                nc.gpsimd.tensor_tensor(out=tsq[:], in0=t_t[:], in1=t_t[:],
                                        op=mybir.AluOpType.mult)
                q_t = stats.tile([128, 16], f32, tag="q")
                nc.vector.tensor_reduce(out=q_t[:], in_=tsq[:],
                                        axis=mybir.AxisListType.X,
                                        op=mybir.AluOpType.add)
                nc.vector.scalar_tensor_tensor(
                    out=var8[:, slot, :], in0=q_t[:], scalar=1.0 / 64,
                    in1=zeros16[:],
                    op0=mybir.AluOpType.mult, op1=mybir.AluOpType.add)

                pend.append((t, t_t, slot))
                if slot == GB - 1 or t == n_macro - 1:
                    flush(var8, rs8)

    nc.compile()
    return nc


def kernel(edge_feats, node_feats, edge_index, edge_rbf,
           W1, b1, W2, b2, W3, b3, ln_gamma, ln_beta):
    global _last_in_maps
    E = edge_feats.shape[0]
    e_core = -(-E // (N_CORES * MACRO)) * MACRO     # per-core edges, padded
    n_macro = e_core // MACRO
    e_pad = e_core * N_CORES

    identity_ln = bool(np.allclose(ln_gamma, 1.0) and np.allclose(ln_beta, 0.0))
    key = (e_core, identity_ln)
    if key not in _cache:
        _cache[key] = _build(e_core, n_macro, identity_ln)
    nc = _cache[key]

    row = np.asarray(edge_index[0]).astype(np.int64)
    col = np.asarray(edge_index[1]).astype(np.int64)
    ef32 = np.asarray(edge_feats, dtype=np.float32)
    nf_bf = np.asarray(node_feats, dtype=np.float32).astype(BF16)
    ef_bf = ef32.astype(BF16)
    rbf_bf = np.asarray(edge_rbf, dtype=np.float32).astype(BF16)

    xa = np.zeros((128, e_pad), dtype=BF16)
    xa[0:64, :E] = nf_bf[row].T
    xa[64:128, :E] = ef_bf.T
    xb = np.zeros((80, e_pad), dtype=BF16)
    xb[0:64, :E] = nf_bf[col].T
    xb[64:80, :E] = rbf_bf.T

    W1f = np.asarray(W1, np.float32)
    w1a = np.ascontiguousarray(
        np.concatenate([W1f[64:128], W1f[0:64]], axis=0)).astype(BF16)
    w1b = np.ascontiguousarray(W1f[128:208]).astype(BF16)
    w2 = np.tile(np.asarray(W2, np.float32).astype(BF16), (2, 1))
    w3 = np.tile(np.asarray(W3, np.float32).astype(BF16), (2, 1))
    iio = np.zeros((128, 65), dtype=np.float32)
    iio[0:64, 0:64] = np.eye(64)
    iio[64:128, 0:64] = np.eye(64)
    iio[:, 64] = 1.0
    iio = iio.astype(BF16)
    brep = np.stack([np.tile(np.asarray(v, np.float32), 2) for v in (b1, b2, b3)],
                    axis=1)                                    # [128, 3]
    gb = np.stack([np.asarray(ln_gamma, np.float32),
                   np.asarray(ln_beta, np.float32)])           # [2, 64]

    in_maps = []
    for c in range(N_CORES):
        s = slice(c * e_core, (c + 1) * e_core)
        in_maps.append({
            "xa": np.ascontiguousarray(xa[:, s]),
            "xb": np.ascontiguousarray(xb[:, s]),
            "w1a": w1a, "w1b": w1b, "w2": w2, "w3": w3,
            "iio": iio, "brep": brep, "gb": gb,
        })

    _last_in_maps = in_maps
    res = run_bass_kernel_spmd(nc, in_maps, core_ids=list(range(N_CORES)))

    out = np.empty((e_pad, 64), dtype=np.float32)
    for c in range(N_CORES):
        blk = res.results[c]["out"].transpose(1, 0, 2).reshape(e_core, 64)
        out[c * e_core:(c + 1) * e_core] = blk
    return out[:E]
